# revision 1
# baseline (speedup 1.0000x reference)
"""GCNII conv kernel for 8 Trainium2 NeuronCores.

Strategy (self-contained; shapes hardcoded):
  - Shard destination nodes across 8 cores (6250 each); edges partitioned by
    destination so each core's segment_sum is local.
  - Host pre-pass: sort edges by dest, group into 128-dest tiles, split each
    tile's edges by source half (int16 gather indices), pad each half to a
    multiple of 128 ("chunks"); per-tile chunk counts are the max over cores
    so all cores run one identical program.
  - Device, per dest tile:
      * dma_gather pulls all the tile's source rows x[row] (512B each) into
        SBUF as [128 edges, chunk, 128 feat] (idx i -> dst[i%128, i//128, :])
      * per chunk, one fused DVE op builds the scaled scatter matrix
        S[e, d] = 0.9*norm[e] * (col_local[e] == d)   (iota==col, then *norm)
      * PE accumulates segT[f, d] += msgs[e, f].T @ S[e, d] in PSUM
      * hT = segT + (0.1*x0).T tile  (alpha folded on host)
      * yT = W_eff @ hT via one matmul, W_eff = (1-beta)*I + beta*W folded on
        host, so no extra elementwise work
  - Output is produced transposed ([128, n_local]) and flipped back on host.
"""

import os
import sys

sys.path.insert(0, "/opt/trn_rl_repo")

import numpy as np

N = 50000
D = 128
NCORES = 8
NPC = N // NCORES          # 6250 dest nodes per core
TPC = (NPC + 127) // 128   # 49 dest tiles per core
NPAD = TPC * 128           # 6272
HALF = N // 2              # int16 gather index split
ALPHA = 0.1
THETA = 0.5
LAYER = 1

_prog_cache = {}

# Stash of the last BassKernelResults for test.py to inspect (exec_time_ns).
LAST = None


def _build_program(schedule):
    """schedule: list of (Mlo, Mhi) per dest tile (shared across cores)."""
    import concourse.bacc as bacc
    import concourse.mybir as mybir
    import concourse.tile as tile
    from concourse import library_config

    f32 = mybir.dt.float32
    bf16 = mybir.dt.bfloat16
    i16 = mybir.dt.int16
    TC = sum(ml + mh for ml, mh in schedule)
    CLO8 = sum(ml for ml, _ in schedule) * 8
    CHI8 = sum(mh for _, mh in schedule) * 8

    nc = bacc.Bacc(
        "TRN2", target_bir_lowering=False, debug=False, num_devices=NCORES
    )
    xlo = nc.dram_tensor("xlo", [HALF, D], bf16, kind="ExternalInput").ap()
    xhi = nc.dram_tensor("xhi", [N - HALF, D], bf16, kind="ExternalInput").ap()
    ilo = nc.dram_tensor("ilo", [128, CLO8], i16, kind="ExternalInput").ap()
    ihi = nc.dram_tensor("ihi", [128, CHI8], i16, kind="ExternalInput").ap()
    cols = nc.dram_tensor("cols", [128, TC], f32, kind="ExternalInput").ap()
    nrm = nc.dram_tensor("nrm", [128, TC], f32, kind="ExternalInput").ap()
    iot = nc.dram_tensor("iot", [128, 128], f32, kind="ExternalInput").ap()
    x0t = nc.dram_tensor("x0t", [D, NPAD], f32, kind="ExternalInput").ap()
    wl = nc.dram_tensor("wl", [D, D], f32, kind="ExternalInput").ap()
    yt = nc.dram_tensor("yt", [D, NPAD], f32, kind="ExternalOutput").ap()

    with tile.TileContext(nc) as tc:
        with (
            tc.tile_pool(name="persist", bufs=1) as pp,
            tc.tile_pool(name="msgs", bufs=3) as mp,
            tc.tile_pool(name="sel", bufs=6) as sp,
            tc.tile_pool(name="hout", bufs=2) as hp,
            tc.tile_pool(name="io", bufs=2) as iop,
            tc.tile_pool(name="pseg", bufs=2, space="PSUM") as psp,
            tc.tile_pool(name="py", bufs=2, space="PSUM") as pyp,
        ):
            nc.gpsimd.load_library(library_config.mlp)

            ilo_sb = pp.tile([128, CLO8], i16)
            ihi_sb = pp.tile([128, CHI8], i16)
            cols_sb = pp.tile([128, TC], f32)
            nrm_sb = pp.tile([128, TC], f32)
            wl_sb = pp.tile([128, 128], f32)
            iota_f = pp.tile([128, 128], f32)

            nc.sync.dma_start(ilo_sb[:], ilo[:, :])
            nc.sync.dma_start(ihi_sb[:], ihi[:, :])
            nc.sync.dma_start(cols_sb[:], cols[:, :])
            nc.sync.dma_start(nrm_sb[:], nrm[:, :])
            nc.sync.dma_start(wl_sb[:], wl[:, :])
            nc.sync.dma_start(iota_f[:], iot[:, :])

            ci = 0
            lo_off = 0
            hi_off = 0
            for t, (Mlo, Mhi) in enumerate(schedule):
                M = Mlo + Mhi
                msgs = mp.tile([128, M, 128], bf16, tag="msgs")
                if Mlo:
                    nc.gpsimd.dma_gather(
                        msgs[:, 0:Mlo, :],
                        xlo[:, :],
                        ilo_sb[:, lo_off * 8 : (lo_off + Mlo) * 8],
                        Mlo * 128,
                        Mlo * 128,
                        D,
                        single_packet=False,
                    )
                if Mhi:
                    nc.gpsimd.dma_gather(
                        msgs[:, Mlo:M, :],
                        xhi[:, :],
                        ihi_sb[:, hi_off * 8 : (hi_off + Mhi) * 8],
                        Mhi * 128,
                        Mhi * 128,
                        D,
                        single_packet=False,
                    )
                ps = psp.tile([128, 128], f32, space="PSUM", tag="pseg")
                for j in range(M):
                    S = sp.tile([128, 128], bf16, tag="sel")
                    nc.vector.tensor_scalar(
                        out=S[:],
                        in0=iota_f[:],
                        scalar1=cols_sb[:, ci + j : ci + j + 1],
                        scalar2=nrm_sb[:, ci + j : ci + j + 1],
                        op0=mybir.AluOpType.is_equal,
                        op1=mybir.AluOpType.mult,
                    )
                    nc.tensor.matmul(
                        ps[:],
                        lhsT=msgs[:, j, :],
                        rhs=S[:],
                        start=(j == 0),
                        stop=(j == M - 1),
                    )
                x0tile = iop.tile([128, 128], f32, tag="x0")
                nc.sync.dma_start(x0tile[:], x0t[:, t * 128 : (t + 1) * 128])
                hT = hp.tile([128, 128], f32, tag="h")
                nc.vector.tensor_tensor(
                    out=hT[:], in0=ps[:], in1=x0tile[:], op=mybir.AluOpType.add
                )
                yp = pyp.tile([128, 128], f32, space="PSUM", tag="py")
                nc.tensor.matmul(
                    yp[:], lhsT=wl_sb[:], rhs=hT[:], start=True, stop=True
                )
                yo = iop.tile([128, 128], f32, tag="yo")
                nc.vector.tensor_copy(yo[:], yp[:])
                nc.sync.dma_start(yt[:, t * 128 : (t + 1) * 128], yo[:])
                ci += M
                lo_off += Mlo
                hi_off += Mhi

    nc.compile()
    return nc


def _wrap16(idx_list):
    """int16 idx list (len = M*128) -> [128, M*8] wrapped+replicated layout:
    idx i is read from partition i%16, free slot i//16; replicate x8."""
    w = idx_list.reshape(-1, 16).T.astype(np.int16)  # [16, M*8]
    return np.tile(w, (8, 1))


def _preprocess(x, x0, edge_index, norm, W):
    row = np.ascontiguousarray(edge_index[0]).astype(np.int64)
    col = np.ascontiguousarray(edge_index[1]).astype(np.int64)
    norm = np.ascontiguousarray(norm).astype(np.float32)
    x = np.ascontiguousarray(x).astype(np.float32)
    x0 = np.ascontiguousarray(x0).astype(np.float32)
    W = np.ascontiguousarray(W).astype(np.float32)

    order = np.argsort(col, kind="stable")
    rs = row[order]
    cs = col[order]
    ns = (1.0 - ALPHA) * norm[order]

    # Global 128-dest tiles, snake-dealt to cores by edge count so per-slot
    # chunk counts are balanced (minimizes shared-schedule padding).
    NT = (N + 127) // 128  # 391
    tstart = np.arange(NT) * 128
    tend = np.minimum(tstart + 128, N)
    e_lo = np.searchsorted(cs, tstart, side="left")
    e_hi = np.searchsorted(cs, tend, side="left")
    cnt = e_hi - e_lo

    order_t = np.argsort(-cnt, kind="stable")
    SLOTS = TPC  # 49 rounds
    assign = -np.ones((NCORES, SLOTS), dtype=np.int64)  # -1 = dummy tile
    k = 0
    for r in range(SLOTS):
        picks = order_t[k : k + NCORES]
        k += len(picks)
        cores = range(NCORES) if r % 2 == 0 else range(NCORES - 1, -1, -1)
        for i, c in enumerate(cores):
            if i < len(picks):
                assign[c, r] = picks[i]

    # Per (core, slot): lo/hi edge lists
    per_ct = {}
    Mlo_ct = np.zeros((NCORES, SLOTS), dtype=np.int64)
    Mhi_ct = np.zeros((NCORES, SLOTS), dtype=np.int64)
    for c in range(NCORES):
        for t in range(SLOTS):
            g = assign[c, t]
            if g < 0:
                per_ct[(c, t)] = None
                continue
            e0, e1 = e_lo[g], e_hi[g]
            r = rs[e0:e1]
            cl = (cs[e0:e1] - tstart[g]).astype(np.float32)
            nn2 = ns[e0:e1]
            m = r < HALF
            per_ct[(c, t)] = (r[m], cl[m], nn2[m], r[~m] - HALF, cl[~m], nn2[~m])
            Mlo_ct[c, t] = -(-int(m.sum()) // 128)
            Mhi_ct[c, t] = -(-int((~m).sum()) // 128)

    Mlo_t = Mlo_ct.max(axis=0)
    Mhi_t = Mhi_ct.max(axis=0)
    zero = (Mlo_t + Mhi_t) == 0
    Mlo_t[zero] = 1
    schedule = [(int(a), int(b)) for a, b in zip(Mlo_t, Mhi_t)]
    TC = int((Mlo_t + Mhi_t).sum())
    CLO = int(Mlo_t.sum())
    CHI = int(Mhi_t.sum())

    beta = np.float32(np.log(THETA / LAYER + 1.0))
    W_eff = (1.0 - beta) * np.eye(D, dtype=np.float32) + beta * W
    wl = np.ascontiguousarray(W_eff.T)
    import ml_dtypes

    bf = ml_dtypes.bfloat16
    iot = np.ascontiguousarray(
        np.tile(np.arange(128, dtype=np.float32)[None, :], (128, 1))
    )
    xlo = np.ascontiguousarray(x[:HALF]).astype(bf)
    xhi = np.ascontiguousarray(x[HALF:]).astype(bf)

    in_maps = []
    for c in range(NCORES):
        ilo_a = np.zeros((128, CLO * 8), dtype=np.int16)
        ihi_a = np.zeros((128, CHI * 8), dtype=np.int16)
        cols_a = np.full((128, TC), -1.0, dtype=np.float32)
        nrm_a = np.zeros((128, TC), dtype=np.float32)
        x0t = np.zeros((D, NPAD), dtype=np.float32)
        ci = 0
        lo_off = 0
        hi_off = 0
        for t in range(SLOTS):
            Mlo, Mhi = int(Mlo_t[t]), int(Mhi_t[t])
            data = per_ct[(c, t)]
            if data is not None:
                rl, cll, nl, rh, clh, nh = data
                g = assign[c, t]
                sz = int(tend[g] - tstart[g])
                x0t[:, t * 128 : t * 128 + sz] = (
                    ALPHA * x0[tstart[g] : tend[g]]
                ).T
            else:
                rl = cll = nl = rh = clh = nh = np.zeros(0)
            for (ri, cli, nni, M, ia, off) in (
                (rl, cll, nl, Mlo, ilo_a, lo_off),
                (rh, clh, nh, Mhi, ihi_a, hi_off),
            ):
                if M == 0:
                    continue
                n_e = len(ri)
                pi = np.zeros(M * 128, dtype=np.int64)
                pc = np.full(M * 128, -1.0, dtype=np.float32)
                pn = np.zeros(M * 128, dtype=np.float32)
                pi[:n_e] = ri
                pc[:n_e] = cli
                pn[:n_e] = nni
                ia[:, off * 8 : (off + M) * 8] = _wrap16(pi)
                cols_a[:, ci : ci + M] = pc.reshape(M, 128).T
                nrm_a[:, ci : ci + M] = pn.reshape(M, 128).T
                ci += M
            lo_off += Mlo
            hi_off += Mhi

        in_maps.append(
            {
                "xlo": xlo,
                "xhi": xhi,
                "ilo": ilo_a,
                "ihi": ihi_a,
                "cols": cols_a,
                "nrm": nrm_a,
                "iot": iot,
                "x0t": np.ascontiguousarray(x0t),
                "wl": wl,
            }
        )
    return schedule, in_maps, (assign, tstart, tend)


def kernel(x, x0, edge_index, norm, W):
    global LAST
    from concourse.bass_utils import run_bass_kernel_spmd

    schedule, in_maps, (assign, tstart, tend) = _preprocess(
        x, x0, edge_index, norm, W
    )
    key = tuple(schedule)
    if key not in _prog_cache:
        _prog_cache[key] = _build_program(schedule)
    nc = _prog_cache[key]

    trace = os.environ.get("KERNEL_TRACE", "0") == "1"
    res = run_bass_kernel_spmd(
        nc,
        in_maps,
        core_ids=list(range(NCORES)),
        trace=trace,
    )
    LAST = res

    y = np.empty((N, D), dtype=np.float32)
    for c in range(NCORES):
        yt = res.results[c]["yt"]
        for t in range(TPC):
            g = assign[c, t]
            if g < 0:
                continue
            sz = int(tend[g] - tstart[g])
            y[tstart[g] : tend[g]] = yt[:, t * 128 : t * 128 + sz].T
    return y



# revision 2
# speedup vs baseline: 1.0189x; 1.0189x over previous
"""GCNII conv kernel v2 for 8 Trainium2 NeuronCores.

Key structure (vs baseline):
  - Dest nodes sharded by range: core c owns cols [c*6250, (c+1)*6250).
  - Dest tiles of width TW=96 within each core; edges per (tile, src-half)
    chunked into 128-edge chunks; schedule shared across cores (max pad).
  - Gathers issued per (group of tiles, half) on 4 SWDGE queues so up to 4
    Q7 pairs generate descriptors concurrently (dma_gather desc-gen is the
    machine bottleneck at ~6-8ns/idx serial per queue).
  - Selection matrices built in batched single-pass custom DVE op:
      S[e, b*TW + d] = (Idx == colp[e,b]) * nrm[e,b]
    (colp pre-offset by TW*batch-pos on host; f32 cols exact).
  - x0 added via identity-matmul into the segment PSUM; y = W_eff @ h via
    wide matmuls over groups of tiles.
"""

import os
import sys

sys.path.insert(0, "/opt/trn_rl_repo")

import numpy as np

N = 50000
D = 128
NCORES = 8
NPC = N // NCORES          # 6250 dest nodes per core
TW = 96                    # dest-tile width
TPC = (NPC + TW - 1) // TW   # 66 tiles per core
NPAD = TPC * TW             # 6336
HALF = N // 2
ALPHA = 0.1
THETA = 0.5
LAYER = 1
NQ = 4                     # SWDGE queues
Q0_WEIGHT = 0.75           # queue 0 shares the dispatcher Q7 pair
TPG = 3                    # tiles per group
KB = 32                    # S-build batch (chunks per custom-DVE op)
USE_CUSTOM_DVE = os.environ.get("KERNEL_NO_CUSTOM", "0") != "1"

_prog_cache = {}
LAST = None
_dve_op = None


def _get_custom_op():
    """Register (once) the fused one-hot*scale DVE op."""
    global _dve_op
    if _dve_op is not None:
        return _dve_op
    import concourse.dve_ops as dve_ops
    from concourse.dve_spec import Spec, Src0, Src1, eq, Idx, lower

    def _ref(in0, in1, c0, c1, c2):
        p = in0.shape[0]
        f0 = in0.reshape(p, -1).astype(np.float32)
        f1 = in1.reshape(p, -1).astype(np.float32)
        idx = np.arange(f0.shape[1], dtype=np.float32)[None, :]
        return (f0 == idx) * f1

    spec = Spec(body=eq(Idx, Src0) * Src1, reference=_ref)
    op = dve_ops.DveOp("ONEHOT_NRM_GCN", spec, subdim=False, uops_sha={})
    # register before sha pinning so opcode lookup works
    if op.name not in dve_ops._SUB_OPCODE_FOR_NAME:
        row = max(dve_ops._SUB_OPCODE_FOR_NAME.values()) + 1
        assert row < 0x20, row
        dve_ops.OPS.append(op)
        dve_ops._SUB_OPCODE_FOR_NAME[op.name] = row
        dve_ops.CUSTOM_DVE_SPECS[op.name] = spec
    # pin shas for both uop versions
    for ver in ("v3", "v4"):
        try:
            uops = lower(spec, ver=ver)
        except Exception:
            continue
        res = dve_ops.DveOpSpec(
            name=op.name,
            opcode=dve_ops.get_dve_sub_opcode(op.name),
            uops=uops,
            rd1_en=True,
        )
        op.uops_sha[ver] = res.sha(ver)
    _dve_op = op
    return op


def _wrap16(idx_list):
    w = idx_list.reshape(-1, 16).T.astype(np.int16)
    return np.tile(w, (8, 1))


def _build_program(schedule):
    """schedule: dict with per-tile chunk counts (shared across cores).

    schedule = {
      'Mlo': [TPC], 'Mhi': [TPC],  # chunks per (tile, half)
      'groups': [(t0, t1), ...],   # tile ranges per group
    }
    """
    import concourse.bacc as bacc
    import concourse.mybir as mybir
    import concourse.tile as tile
    from concourse import library_config

    f32 = mybir.dt.float32
    bf16 = mybir.dt.bfloat16
    i16 = mybir.dt.int16

    Mlo = schedule["Mlo"]
    Mhi = schedule["Mhi"]
    groups = schedule["groups"]
    TC = int(sum(Mlo) + sum(Mhi))
    CLO = int(sum(Mlo))
    CHI = int(sum(Mhi))

    op = _get_custom_op() if USE_CUSTOM_DVE else None

    nc = bacc.Bacc(
        "TRN2", target_bir_lowering=False, debug=False,
        num_devices=NCORES, num_swdge_queues=NQ,
    )
    xlo = nc.dram_tensor("xlo", [HALF, D], bf16, kind="ExternalInput").ap()
    xhi = nc.dram_tensor("xhi", [N - HALF, D], bf16, kind="ExternalInput").ap()
    ilo = nc.dram_tensor("ilo", [128, CLO * 8], i16, kind="ExternalInput").ap()
    ihi = nc.dram_tensor("ihi", [128, CHI * 8], i16, kind="ExternalInput").ap()
    colp = nc.dram_tensor("colp", [128, TC], f32, kind="ExternalInput").ap()
    nrm = nc.dram_tensor("nrm", [128, TC], f32, kind="ExternalInput").ap()
    iot = nc.dram_tensor("iot", [128, TW], bf16, kind="ExternalInput").ap()
    x0t = nc.dram_tensor("x0t", [D, NPAD], bf16, kind="ExternalInput").ap()
    wl = nc.dram_tensor("wl", [D, D], bf16, kind="ExternalInput").ap()
    idm = nc.dram_tensor("idm", [D, D], bf16, kind="ExternalInput").ap()
    yt = nc.dram_tensor("yt", [D, NPAD], f32, kind="ExternalOutput").ap()

    with tile.TileContext(nc) as tc:
        with (
            tc.tile_pool(name="persist", bufs=1) as pp,
            tc.tile_pool(name="msl", bufs=6) as mpl,
            tc.tile_pool(name="msh", bufs=6) as mph,
            tc.tile_pool(name="sel", bufs=6) as sp,
            tc.tile_pool(name="hg", bufs=3) as hp,
            tc.tile_pool(name="x0g", bufs=3) as xp,
            tc.tile_pool(name="yg", bufs=2) as yp_pool,
            tc.tile_pool(name="pseg", bufs=6, space="PSUM") as psp,
            tc.tile_pool(name="py", bufs=2, space="PSUM") as pyp,
        ):
            nc.gpsimd.load_library(library_config.mlp)

            colp_sb = pp.tile([128, TC], f32)
            nrm_sb = pp.tile([128, TC], f32)
            wl_sb = pp.tile([128, 128], bf16)
            idm_sb = pp.tile([128, 128], bf16)
            iota_sb = pp.tile([128, TW], bf16)

            nc.sync.dma_start(colp_sb[:], colp[:, :])
            nc.sync.dma_start(nrm_sb[:], nrm[:, :])
            nc.sync.dma_start(wl_sb[:], wl[:, :])
            nc.sync.dma_start(idm_sb[:], idm[:, :])
            nc.sync.dma_start(iota_sb[:], iot[:, :])

            # offsets
            lo_base = np.concatenate([[0], np.cumsum(Mlo)]).astype(int)
            hi_base = np.concatenate([[0], np.cumsum(Mhi)]).astype(int)

            # size-greedy queue assignment (queue 0 weighted down, never
            # first): balances per-queue desc-gen chains
            calls = []
            for gi, (g0, g1) in enumerate(groups):
                calls.append((gi, 0, int(lo_base[g1] - lo_base[g0])))
                calls.append((gi, 1, int(hi_base[g1] - hi_base[g0])))
            qload = [0.0] * NQ
            qweight = [Q0_WEIGHT] + [1.0] * (NQ - 1)
            qassign = {}
            for (gi, hf, m) in calls:
                q = min(range(NQ), key=lambda i: qload[i] / qweight[i])
                qassign[(gi, hf)] = q
                qload[q] += m

            # per-group idx tiles (separate tiles -> fine-grained DMA deps)
            ilo_g_sb = []
            ihi_g_sb = []
            for (g0, g1) in groups:
                Mlo_g = int(lo_base[g1] - lo_base[g0])
                Mhi_g = int(hi_base[g1] - hi_base[g0])
                tl = pp.tile([128, max(Mlo_g, 1) * 8], i16, name=f"ilo_g{g0}")
                th = pp.tile([128, max(Mhi_g, 1) * 8], i16, name=f"ihi_g{g0}")
                if Mlo_g:
                    nc.sync.dma_start(
                        tl[:], ilo[:, lo_base[g0] * 8 : lo_base[g1] * 8]
                    )
                if Mhi_g:
                    nc.sync.dma_start(
                        th[:], ihi[:, hi_base[g0] * 8 : hi_base[g1] * 8]
                    )
                ilo_g_sb.append(tl)
                ihi_g_sb.append(th)

            # consumption order S-column index per (tile): lo chunks then hi
            ci_of_tile = {}
            ci = 0
            for t in range(TPC):
                ci_of_tile[t] = ci
                ci += int(Mlo[t]) + int(Mhi[t])
            assert ci == TC

            alt = [0]  # alternate copies between DVE and Act

            for gi, (g0, g1) in enumerate(groups):
                ng = g1 - g0
                Mlo_g = int(lo_base[g1] - lo_base[g0])
                Mhi_g = int(hi_base[g1] - hi_base[g0])

                x0g = xp.tile([128, ng * TW], bf16, tag="x0")
                nc.sync.dma_start(x0g[:], x0t[:, g0 * TW : g1 * TW])

                msl = mpl.tile([128, max(Mlo_g, 1), 128], bf16, tag="msl")
                if Mlo_g:
                    nc.gpsimd.dma_gather(
                        msl[:, :, :],
                        xlo[:, :],
                        ilo_g_sb[gi][:, :],
                        Mlo_g * 128,
                        Mlo_g * 128,
                        D,
                        single_packet=False,
                        queue_num=qassign[(gi, 0)],
                    )
                msh = mph.tile([128, max(Mhi_g, 1), 128], bf16, tag="msh")
                if Mhi_g:
                    nc.gpsimd.dma_gather(
                        msh[:, :, :],
                        xhi[:, :],
                        ihi_g_sb[gi][:, :],
                        Mhi_g * 128,
                        Mhi_g * 128,
                        D,
                        single_packet=False,
                        queue_num=qassign[(gi, 1)],
                    )

                # S-batches over the group's consumption-order column range
                ci0 = ci_of_tile[g0]
                ci1 = ci_of_tile[g1 - 1] + int(Mlo[g1 - 1]) + int(Mhi[g1 - 1])
                n_cols = ci1 - ci0
                s_tiles = {}
                for b0 in range(0, n_cols, KB):
                    bk = min(KB, n_cols - b0)
                    Sb = sp.tile([128, bk * TW], bf16, tag="sel")
                    if op is not None:
                        nc.vector._custom_dve(
                            op,
                            out=Sb[:].rearrange("p (b w) -> p b w", w=TW),
                            in0=colp_sb[:, ci0 + b0 : ci0 + b0 + bk, None]
                            .to_broadcast((128, bk, TW)),
                            in1=nrm_sb[:, ci0 + b0 : ci0 + b0 + bk, None]
                            .to_broadcast((128, bk, TW)),
                        )
                    else:
                        nc.vector.tensor_tensor(
                            out=Sb[:].rearrange("p (b w) -> p b w", w=TW),
                            in0=iot[None] if False else iota_sb[:, None, :]
                            .to_broadcast((128, bk, TW)),
                            in1=colp_sb[:, ci0 + b0 : ci0 + b0 + bk, None]
                            .to_broadcast((128, bk, TW)),
                            op=mybir.AluOpType.is_equal,
                        )
                        nc.vector.tensor_tensor(
                            out=Sb[:].rearrange("p (b w) -> p b w", w=TW),
                            in0=Sb[:].rearrange("p (b w) -> p b w", w=TW),
                            in1=nrm_sb[:, ci0 + b0 : ci0 + b0 + bk, None]
                            .to_broadcast((128, bk, TW)),
                            op=mybir.AluOpType.mult,
                        )
                    for k in range(bk):
                        s_tiles[ci0 + b0 + k] = (Sb, k)

                hgt = hp.tile([128, ng * TW], bf16, tag="hg")
                for t in range(g0, g1):
                    ps = psp.tile([128, TW], f32, space="PSUM", tag="ps")
                    ci_t = ci_of_tile[t]
                    nch = int(Mlo[t]) + int(Mhi[t])
                    for j in range(nch):
                        if j < int(Mlo[t]):
                            src = msl[:, lo_base[t] - lo_base[g0] + j, :]
                        else:
                            jj = j - int(Mlo[t])
                            src = msh[:, hi_base[t] - hi_base[g0] + jj, :]
                        Sb, k = s_tiles[ci_t + j]
                        nc.tensor.matmul(
                            ps[:],
                            lhsT=src,
                            rhs=Sb[:, k * TW : (k + 1) * TW],
                            start=(j == 0),
                            stop=False,
                        )
                    nc.tensor.matmul(
                        ps[:],
                        lhsT=idm_sb[:],
                        rhs=x0g[:, (t - g0) * TW : (t - g0 + 1) * TW],
                        start=(nch == 0),
                        stop=True,
                    )
                    # copy psum -> hg slice (alternate engine)
                    dst = hgt[:, (t - g0) * TW : (t - g0 + 1) * TW]
                    if alt[0] % 2 == 0:
                        nc.vector.tensor_copy(dst, ps[:])
                    else:
                        nc.scalar.copy(dst, ps[:])
                    alt[0] += 1

                # TW matmul over the whole group (ng*TW <= 512)
                ygt = yp_pool.tile([128, ng * TW], f32, tag="yg")
                pyt = pyp.tile([128, ng * TW], f32, space="PSUM", tag="py")
                nc.tensor.matmul(
                    pyt[:], lhsT=wl_sb[:], rhs=hgt[:], start=True, stop=True
                )
                if alt[0] % 2 == 0:
                    nc.vector.tensor_copy(ygt[:], pyt[:])
                else:
                    nc.scalar.copy(ygt[:], pyt[:])
                alt[0] += 1
                nc.sync.dma_start(yt[:, g0 * TW : g1 * TW], ygt[:])

    nc.compile()
    return nc


def _preprocess(x, x0, edge_index, norm, Wm):
    row = np.ascontiguousarray(edge_index[0]).astype(np.int64)
    col = np.ascontiguousarray(edge_index[1]).astype(np.int64)
    norm = np.ascontiguousarray(norm).astype(np.float32)
    x = np.ascontiguousarray(x).astype(np.float32)
    x0 = np.ascontiguousarray(x0).astype(np.float32)
    Wm = np.ascontiguousarray(Wm).astype(np.float32)

    import ml_dtypes

    bf = ml_dtypes.bfloat16

    core = col // NPC
    tloc = (col % NPC) // TW
    is_hi = row >= HALF
    nscaled = (1.0 - ALPHA) * norm

    # sort by (core, tile, half, col) -- stable ordering for chunking
    order = np.lexsort((col, is_hi, tloc, core))
    rs = row[order]
    cs = col[order]
    ns = nscaled[order]
    core_s = core[order]
    t_s = tloc[order]
    h_s = is_hi[order]

    # per (core, tile, half) edge count
    key = (core_s * TPC + t_s) * 2 + h_s
    cnt = np.bincount(key, minlength=NCORES * TPC * 2).reshape(NCORES, TPC, 2)
    Mct = -(-cnt // 128)  # ceil chunks per (core, tile, half)
    Mlo = Mct[:, :, 0].max(axis=0)
    Mhi = Mct[:, :, 1].max(axis=0)

    # groups of TPG tiles
    groups = [(g0, min(g0 + TPG, TPC)) for g0 in range(0, TPC, TPG)]

    schedule = {
        "Mlo": tuple(int(v) for v in Mlo),
        "Mhi": tuple(int(v) for v in Mhi),
        "groups": tuple(groups),
    }

    CLO = int(Mlo.sum())
    CHI = int(Mhi.sum())
    TC = CLO + CHI

    beta = np.float32(np.log(THETA / LAYER + 1.0))
    W_eff = (1.0 - beta) * np.eye(D, dtype=np.float32) + beta * Wm
    wl = np.ascontiguousarray(W_eff.T).astype(bf)
    idm = np.eye(D, dtype=np.float32).astype(bf)
    iot = np.ascontiguousarray(
        np.tile(np.arange(TW, dtype=np.float32)[None, :], (128, 1))
    ).astype(bf)
    xlo = np.ascontiguousarray(x[:HALF]).astype(bf)
    xhi = np.ascontiguousarray(x[HALF:]).astype(bf)

    # boundaries of each (core,tile,half) run in the sorted arrays
    starts = np.zeros(NCORES * TPC * 2 + 1, dtype=np.int64)
    np.cumsum(cnt.reshape(-1), out=starts[1:])

    lo_base = np.concatenate([[0], np.cumsum(Mlo)]).astype(int)
    hi_base = np.concatenate([[0], np.cumsum(Mhi)]).astype(int)

    # consumption-order column base per tile
    ci_of_tile = np.zeros(TPC + 1, dtype=np.int64)
    np.cumsum(Mlo + Mhi, out=ci_of_tile[1:])

    in_maps = []
    for c in range(NCORES):
        ilo_a = np.zeros((128, CLO * 8), dtype=np.int16)
        ihi_a = np.zeros((128, CHI * 8), dtype=np.int16)
        colp_a = np.full((128, TC), -1.0, dtype=np.float32)
        nrm_a = np.zeros((128, TC), dtype=np.float32)
        x0t = np.zeros((D, NPAD), dtype=np.float32)
        d0 = c * NPC
        d1 = min(d0 + NPC, N)
        x0t[:, : d1 - d0] = (ALPHA * x0[d0:d1]).T

        for t in range(TPC):
            for hf, (M, ia, base) in enumerate(
                ((int(Mlo[t]), ilo_a, lo_base[t]), (int(Mhi[t]), ihi_a, hi_base[t]))
            ):
                if M == 0:
                    continue
                k = (c * TPC + t) * 2 + hf
                e0, e1 = int(starts[k]), int(starts[k + 1])
                n_e = e1 - e0
                pi = np.zeros(M * 128, dtype=np.int64)
                pc = np.full(M * 128, -1.0, dtype=np.float32)
                pn = np.zeros(M * 128, dtype=np.float32)
                pi[:n_e] = rs[e0:e1] - (HALF if hf else 0)
                pc[:n_e] = (cs[e0:e1] - d0 - t * TW).astype(np.float32)
                pn[:n_e] = ns[e0:e1]
                ia[:, base * 8 : (base + M) * 8] = _wrap16(pi)
                # consumption-order columns: tile t chunk j -> ci_of_tile[t]+ (hf? Mlo[t]:0) + j
                cbase = int(ci_of_tile[t]) + (int(Mlo[t]) if hf else 0)
                colp_a[:, cbase : cbase + M] = pc.reshape(M, 128).T
                nrm_a[:, cbase : cbase + M] = pn.reshape(M, 128).T

        # add TW*batch-position offsets for the fused Idx comparison
        if USE_CUSTOM_DVE:
            # batches are per group, KB chunks each, offsets = TW * (pos in batch)
            for (g0, g1) in groups:
                ci0 = int(ci_of_tile[g0])
                ci1 = int(ci_of_tile[g1])
                n_cols = ci1 - ci0
                for b0 in range(0, n_cols, KB):
                    bk = min(KB, n_cols - b0)
                    offs = (np.arange(bk) * TW).astype(np.float32)
                    blk = colp_a[:, ci0 + b0 : ci0 + b0 + bk]
                    # padded (-1) stays out of range: -1 + TW*k never equals
                    # Idx in [TW*k, TW*(k+1)) except -1+TW*(k+1) = TW*k+TW-1-TW...
                    # careful: -1 + offs_next could alias; keep pads at -1e9
                    blk[blk < 0] = -1e9
                    blk += offs[None, :]
                    colp_a[:, ci0 + b0 : ci0 + b0 + bk] = blk

        in_maps.append(
            {
                "xlo": xlo,
                "xhi": xhi,
                "ilo": ilo_a,
                "ihi": ihi_a,
                "colp": colp_a,
                "nrm": nrm_a,
                "iot": iot,
                "x0t": np.ascontiguousarray(x0t).astype(bf),
                "wl": wl,
                "idm": idm,
                "yt": None,
            }
        )
        in_maps[-1].pop("yt")
    return schedule, in_maps


def kernel(x, x0, edge_index, norm, W):
    global LAST
    from concourse.bass_utils import run_bass_kernel_spmd

    schedule, in_maps = _preprocess(x, x0, edge_index, norm, W)
    key = (schedule["Mlo"], schedule["Mhi"], schedule["groups"])
    if key not in _prog_cache:
        _prog_cache[key] = _build_program(schedule)
    nc = _prog_cache[key]

    trace = os.environ.get("KERNEL_TRACE", "0") == "1"
    res = run_bass_kernel_spmd(
        nc,
        in_maps,
        core_ids=list(range(NCORES)),
        trace=trace,
    )
    LAST = res

    y = np.empty((N, D), dtype=np.float32)
    for c in range(NCORES):
        ytc = res.results[c]["yt"]
        d0 = c * NPC
        d1 = min(d0 + NPC, N)
        y[d0:d1] = ytc[:, : d1 - d0].T
    return y


# keep the reference-compatible signature name `TW` for kwargs call
def kernel_entry(**inputs):
    return kernel(
        inputs["x"], inputs["x0"], inputs["edge_index"], inputs["norm"], inputs["W"]
    )


# revision 3
# speedup vs baseline: 1.1408x; 1.1196x over previous
"""GCNII conv kernel v2 for 8 Trainium2 NeuronCores.

Key structure (vs baseline):
  - Dest nodes sharded by range: core c owns cols [c*6250, (c+1)*6250).
  - Dest tiles of width TW=96 within each core; edges per (tile, src-half)
    chunked into 128-edge chunks; schedule shared across cores (max pad).
  - Gathers issued per (group of tiles, half) on 4 SWDGE queues so up to 4
    Q7 pairs generate descriptors concurrently (dma_gather desc-gen is the
    machine bottleneck at ~6-8ns/idx serial per queue).
  - Selection matrices built in batched single-pass custom DVE op:
      S[e, b*TW + d] = (Idx == colp[e,b]) * nrm[e,b]
    (colp pre-offset by TW*batch-pos on host; f32 cols exact).
  - x0 added via identity-matmul into the segment PSUM; y = W_eff @ h via
    wide matmuls over groups of tiles.
"""

import os
import sys

sys.path.insert(0, "/opt/trn_rl_repo")

import numpy as np

N = 50000
D = 128
NCORES = 8
NPC = N // NCORES          # 6250 dest nodes per core
TW = 96                    # dest-tile width
TPC = (NPC + TW - 1) // TW   # 66 tiles per core
NPAD = TPC * TW             # 6336
HALF = N // 2
ALPHA = 0.1
THETA = 0.5
LAYER = 1
NQ = 4                     # SWDGE queues
Q0_WEIGHT = 0.75           # queue 0 shares the dispatcher Q7 pair
TPG = 3                    # tiles per group
KB = 32                    # S-build batch (chunks per custom-DVE op)
USE_CUSTOM_DVE = os.environ.get("KERNEL_NO_CUSTOM", "0") != "1"

_prog_cache = {}
LAST = None
_dve_op = None


def _get_custom_op():
    """Register (once) the fused one-hot*scale DVE op."""
    global _dve_op
    if _dve_op is not None:
        return _dve_op
    import concourse.dve_ops as dve_ops
    from concourse.dve_spec import Spec, Src0, Src1, eq, Idx, lower

    def _ref(in0, in1, c0, c1, c2):
        p = in0.shape[0]
        f0 = in0.reshape(p, -1).astype(np.float32)
        f1 = in1.reshape(p, -1).astype(np.float32)
        idx = np.arange(f0.shape[1], dtype=np.float32)[None, :]
        return (f0 == idx) * f1

    spec = Spec(body=eq(Idx, Src0) * Src1, reference=_ref)
    op = dve_ops.DveOp("ONEHOT_NRM_GCN", spec, subdim=False, uops_sha={})
    # register before sha pinning so opcode lookup works
    if op.name not in dve_ops._SUB_OPCODE_FOR_NAME:
        row = max(dve_ops._SUB_OPCODE_FOR_NAME.values()) + 1
        assert row < 0x20, row
        dve_ops.OPS.append(op)
        dve_ops._SUB_OPCODE_FOR_NAME[op.name] = row
        dve_ops.CUSTOM_DVE_SPECS[op.name] = spec
    # pin shas for both uop versions
    for ver in ("v3", "v4"):
        try:
            uops = lower(spec, ver=ver)
        except Exception:
            continue
        res = dve_ops.DveOpSpec(
            name=op.name,
            opcode=dve_ops.get_dve_sub_opcode(op.name),
            uops=uops,
            rd1_en=True,
        )
        op.uops_sha[ver] = res.sha(ver)
    _dve_op = op
    return op


def _wrap16(idx_list):
    w = idx_list.reshape(-1, 16).T.astype(np.int16)
    return np.tile(w, (8, 1))


def _build_program(schedule):
    """schedule: dict with per-tile chunk counts (shared across cores).

    schedule = {
      'Mlo': [TPC], 'Mhi': [TPC],  # chunks per (tile, half)
      'groups': [(t0, t1), ...],   # tile ranges per group
    }
    """
    import concourse.bacc as bacc
    import concourse.mybir as mybir
    import concourse.tile as tile
    from concourse import library_config

    f32 = mybir.dt.float32
    bf16 = mybir.dt.bfloat16
    i16 = mybir.dt.int16

    Mlo = schedule["Mlo"]
    Mhi = schedule["Mhi"]
    groups = schedule["groups"]
    TC = int(sum(Mlo) + sum(Mhi))
    CLO = int(sum(Mlo))
    CHI = int(sum(Mhi))

    op = _get_custom_op() if USE_CUSTOM_DVE else None

    nc = bacc.Bacc(
        "TRN2", target_bir_lowering=False, debug=False,
        num_devices=NCORES, num_swdge_queues=NQ,
    )
    xlo = nc.dram_tensor("xlo", [HALF, D], bf16, kind="ExternalInput").ap()
    xhi = nc.dram_tensor("xhi", [N - HALF, D], bf16, kind="ExternalInput").ap()
    ilo = nc.dram_tensor("ilo", [128, CLO * 8], i16, kind="ExternalInput").ap()
    ihi = nc.dram_tensor("ihi", [128, CHI * 8], i16, kind="ExternalInput").ap()
    colp = nc.dram_tensor("colp", [128, TC], f32, kind="ExternalInput").ap()
    nrm = nc.dram_tensor("nrm", [128, TC], f32, kind="ExternalInput").ap()
    iot = nc.dram_tensor("iot", [128, TW], bf16, kind="ExternalInput").ap()
    x0t = nc.dram_tensor("x0t", [D, NPAD], bf16, kind="ExternalInput").ap()
    wl = nc.dram_tensor("wl", [D, D], bf16, kind="ExternalInput").ap()
    idm = nc.dram_tensor("idm", [D, D], bf16, kind="ExternalInput").ap()
    yt = nc.dram_tensor("yt", [D, NPAD], f32, kind="ExternalOutput").ap()

    with tile.TileContext(nc) as tc:
        with (
            tc.tile_pool(name="persist", bufs=1) as pp,
            tc.tile_pool(name="msl", bufs=6) as mpl,
            tc.tile_pool(name="msh", bufs=6) as mph,
            tc.tile_pool(name="sel", bufs=6) as sp,
            tc.tile_pool(name="hg", bufs=3) as hp,
            tc.tile_pool(name="x0g", bufs=3) as xp,
            tc.tile_pool(name="yg", bufs=2) as yp_pool,
            tc.tile_pool(name="pseg", bufs=6, space="PSUM") as psp,
            tc.tile_pool(name="py", bufs=2, space="PSUM") as pyp,
        ):
            nc.gpsimd.load_library(library_config.mlp)

            colp_sb = pp.tile([128, TC], f32)
            nrm_sb = pp.tile([128, TC], f32)
            wl_sb = pp.tile([128, 128], bf16)
            idm_sb = pp.tile([128, 128], bf16)
            iota_sb = pp.tile([128, TW], bf16)

            nc.sync.dma_start(colp_sb[:], colp[:, :])
            nc.sync.dma_start(nrm_sb[:], nrm[:, :])
            nc.sync.dma_start(wl_sb[:], wl[:, :])
            nc.sync.dma_start(idm_sb[:], idm[:, :])
            nc.sync.dma_start(iota_sb[:], iot[:, :])

            # offsets
            lo_base = np.concatenate([[0], np.cumsum(Mlo)]).astype(int)
            hi_base = np.concatenate([[0], np.cumsum(Mhi)]).astype(int)

            # size-greedy queue assignment (queue 0 weighted down, never
            # first): balances per-queue desc-gen chains
            calls = []
            for gi, (g0, g1) in enumerate(groups):
                calls.append((gi, 0, int(lo_base[g1] - lo_base[g0])))
                calls.append((gi, 1, int(hi_base[g1] - hi_base[g0])))
            qload = [0.0] * NQ
            qweight = [Q0_WEIGHT] + [1.0] * (NQ - 1)
            qassign = {}
            for (gi, hf, m) in calls:
                q = min(range(NQ), key=lambda i: qload[i] / qweight[i])
                qassign[(gi, hf)] = q
                qload[q] += m

            # per-group idx tiles (separate tiles -> fine-grained DMA deps)
            ilo_g_sb = []
            ihi_g_sb = []
            for (g0, g1) in groups:
                Mlo_g = int(lo_base[g1] - lo_base[g0])
                Mhi_g = int(hi_base[g1] - hi_base[g0])
                tl = pp.tile([128, max(Mlo_g, 1) * 8], i16, name=f"ilo_g{g0}")
                th = pp.tile([128, max(Mhi_g, 1) * 8], i16, name=f"ihi_g{g0}")
                if Mlo_g:
                    nc.sync.dma_start(
                        tl[:], ilo[:, lo_base[g0] * 8 : lo_base[g1] * 8]
                    )
                if Mhi_g:
                    nc.sync.dma_start(
                        th[:], ihi[:, hi_base[g0] * 8 : hi_base[g1] * 8]
                    )
                ilo_g_sb.append(tl)
                ihi_g_sb.append(th)

            # consumption order S-column index per (tile): lo chunks then hi
            ci_of_tile = {}
            ci = 0
            for t in range(TPC):
                ci_of_tile[t] = ci
                ci += int(Mlo[t]) + int(Mhi[t])
            assert ci == TC

            alt = [0]  # alternate copies between DVE and Act

            for gi, (g0, g1) in enumerate(groups):
                ng = g1 - g0
                Mlo_g = int(lo_base[g1] - lo_base[g0])
                Mhi_g = int(hi_base[g1] - hi_base[g0])

                x0g = xp.tile([128, ng * TW], bf16, tag="x0")
                nc.sync.dma_start(x0g[:], x0t[:, g0 * TW : g1 * TW])

                msl = mpl.tile([128, max(Mlo_g, 1), 128], bf16, tag="msl")
                if Mlo_g:
                    nc.gpsimd.dma_gather(
                        msl[:, :, :],
                        xlo[:, :],
                        ilo_g_sb[gi][:, :],
                        Mlo_g * 128,
                        Mlo_g * 128,
                        D,
                        single_packet=False,
                        queue_num=qassign[(gi, 0)],
                    )
                msh = mph.tile([128, max(Mhi_g, 1), 128], bf16, tag="msh")
                if Mhi_g:
                    nc.gpsimd.dma_gather(
                        msh[:, :, :],
                        xhi[:, :],
                        ihi_g_sb[gi][:, :],
                        Mhi_g * 128,
                        Mhi_g * 128,
                        D,
                        single_packet=False,
                        queue_num=qassign[(gi, 1)],
                    )

                # S-batches over the group's consumption-order column range
                ci0 = ci_of_tile[g0]
                ci1 = ci_of_tile[g1 - 1] + int(Mlo[g1 - 1]) + int(Mhi[g1 - 1])
                n_cols = ci1 - ci0
                s_tiles = {}
                for b0 in range(0, n_cols, KB):
                    bk = min(KB, n_cols - b0)
                    Sb = sp.tile([128, bk * TW], bf16, tag="sel")
                    if op is not None:
                        nc.vector._custom_dve(
                            op,
                            out=Sb[:].rearrange("p (b w) -> p b w", w=TW),
                            in0=colp_sb[:, ci0 + b0 : ci0 + b0 + bk, None]
                            .to_broadcast((128, bk, TW)),
                            in1=nrm_sb[:, ci0 + b0 : ci0 + b0 + bk, None]
                            .to_broadcast((128, bk, TW)),
                        )
                    else:
                        nc.vector.tensor_tensor(
                            out=Sb[:].rearrange("p (b w) -> p b w", w=TW),
                            in0=iot[None] if False else iota_sb[:, None, :]
                            .to_broadcast((128, bk, TW)),
                            in1=colp_sb[:, ci0 + b0 : ci0 + b0 + bk, None]
                            .to_broadcast((128, bk, TW)),
                            op=mybir.AluOpType.is_equal,
                        )
                        nc.vector.tensor_tensor(
                            out=Sb[:].rearrange("p (b w) -> p b w", w=TW),
                            in0=Sb[:].rearrange("p (b w) -> p b w", w=TW),
                            in1=nrm_sb[:, ci0 + b0 : ci0 + b0 + bk, None]
                            .to_broadcast((128, bk, TW)),
                            op=mybir.AluOpType.mult,
                        )
                    for k in range(bk):
                        s_tiles[ci0 + b0 + k] = (Sb, k)

                hgt = hp.tile([128, ng * TW], bf16, tag="hg")
                for t in range(g0, g1):
                    ps = psp.tile([128, TW], f32, space="PSUM", tag="ps")
                    ci_t = ci_of_tile[t]
                    nch = int(Mlo[t]) + int(Mhi[t])
                    for j in range(nch):
                        if j < int(Mlo[t]):
                            src = msl[:, lo_base[t] - lo_base[g0] + j, :]
                        else:
                            jj = j - int(Mlo[t])
                            src = msh[:, hi_base[t] - hi_base[g0] + jj, :]
                        Sb, k = s_tiles[ci_t + j]
                        nc.tensor.matmul(
                            ps[:],
                            lhsT=src,
                            rhs=Sb[:, k * TW : (k + 1) * TW],
                            start=(j == 0),
                            stop=False,
                        )
                    nc.tensor.matmul(
                        ps[:],
                        lhsT=idm_sb[:],
                        rhs=x0g[:, (t - g0) * TW : (t - g0 + 1) * TW],
                        start=(nch == 0),
                        stop=True,
                    )
                    # copy psum -> hg slice (alternate engine)
                    dst = hgt[:, (t - g0) * TW : (t - g0 + 1) * TW]
                    if alt[0] % 2 == 0:
                        nc.vector.tensor_copy(dst, ps[:])
                    else:
                        nc.scalar.copy(dst, ps[:])
                    alt[0] += 1

                # TW matmul over the whole group (ng*TW <= 512)
                ygt = yp_pool.tile([128, ng * TW], f32, tag="yg")
                pyt = pyp.tile([128, ng * TW], f32, space="PSUM", tag="py")
                nc.tensor.matmul(
                    pyt[:], lhsT=wl_sb[:], rhs=hgt[:], start=True, stop=True
                )
                if alt[0] % 2 == 0:
                    nc.vector.tensor_copy(ygt[:], pyt[:])
                else:
                    nc.scalar.copy(ygt[:], pyt[:])
                alt[0] += 1
                nc.sync.dma_start(yt[:, g0 * TW : g1 * TW], ygt[:])

    nc.compile()
    return nc


def _preprocess(x, x0, edge_index, norm, Wm):
    row = np.ascontiguousarray(edge_index[0]).astype(np.int64)
    col = np.ascontiguousarray(edge_index[1]).astype(np.int64)
    norm = np.ascontiguousarray(norm).astype(np.float32)
    x = np.ascontiguousarray(x).astype(np.float32)
    x0 = np.ascontiguousarray(x0).astype(np.float32)
    Wm = np.ascontiguousarray(Wm).astype(np.float32)

    import ml_dtypes

    bf = ml_dtypes.bfloat16

    core = col // NPC
    tloc = (col % NPC) // TW
    is_hi = row >= HALF
    nscaled = (1.0 - ALPHA) * norm

    # sort by (core, tile, half, col) -- stable ordering for chunking
    order = np.lexsort((col, is_hi, tloc, core))
    rs = row[order]
    cs = col[order]
    ns = nscaled[order]
    core_s = core[order]
    t_s = tloc[order]
    h_s = is_hi[order]

    # per (core, tile, half) edge count
    key = (core_s * TPC + t_s) * 2 + h_s
    cnt = np.bincount(key, minlength=NCORES * TPC * 2).reshape(NCORES, TPC, 2)
    Mct = -(-cnt // 128)  # ceil chunks per (core, tile, half)
    Mlo = Mct[:, :, 0].max(axis=0)
    Mhi = Mct[:, :, 1].max(axis=0)

    # groups of TPG tiles
    # taper: tiny first group (fast ramp: first gather's desc-gen blocks
    # dispatch) and tiny last group (short tail)
    gb = [0, 1]
    while gb[-1] < TPC:
        gb.append(min(gb[-1] + TPG, TPC))
    if gb[-1] - gb[-2] > 2:
        gb.insert(-1, gb[-1] - 1)
    groups = list(zip(gb[:-1], gb[1:]))

    schedule = {
        "Mlo": tuple(int(v) for v in Mlo),
        "Mhi": tuple(int(v) for v in Mhi),
        "groups": tuple(groups),
    }

    CLO = int(Mlo.sum())
    CHI = int(Mhi.sum())
    TC = CLO + CHI

    beta = np.float32(np.log(THETA / LAYER + 1.0))
    W_eff = (1.0 - beta) * np.eye(D, dtype=np.float32) + beta * Wm
    wl = np.ascontiguousarray(W_eff.T).astype(bf)
    idm = np.eye(D, dtype=np.float32).astype(bf)
    iot = np.ascontiguousarray(
        np.tile(np.arange(TW, dtype=np.float32)[None, :], (128, 1))
    ).astype(bf)
    xlo = np.ascontiguousarray(x[:HALF]).astype(bf)
    xhi = np.ascontiguousarray(x[HALF:]).astype(bf)

    # boundaries of each (core,tile,half) run in the sorted arrays
    starts = np.zeros(NCORES * TPC * 2 + 1, dtype=np.int64)
    np.cumsum(cnt.reshape(-1), out=starts[1:])

    lo_base = np.concatenate([[0], np.cumsum(Mlo)]).astype(int)
    hi_base = np.concatenate([[0], np.cumsum(Mhi)]).astype(int)

    # consumption-order column base per tile
    ci_of_tile = np.zeros(TPC + 1, dtype=np.int64)
    np.cumsum(Mlo + Mhi, out=ci_of_tile[1:])

    in_maps = []
    for c in range(NCORES):
        ilo_a = np.zeros((128, CLO * 8), dtype=np.int16)
        ihi_a = np.zeros((128, CHI * 8), dtype=np.int16)
        colp_a = np.full((128, TC), -1.0, dtype=np.float32)
        nrm_a = np.zeros((128, TC), dtype=np.float32)
        x0t = np.zeros((D, NPAD), dtype=np.float32)
        d0 = c * NPC
        d1 = min(d0 + NPC, N)
        x0t[:, : d1 - d0] = (ALPHA * x0[d0:d1]).T

        for t in range(TPC):
            for hf, (M, ia, base) in enumerate(
                ((int(Mlo[t]), ilo_a, lo_base[t]), (int(Mhi[t]), ihi_a, hi_base[t]))
            ):
                if M == 0:
                    continue
                k = (c * TPC + t) * 2 + hf
                e0, e1 = int(starts[k]), int(starts[k + 1])
                n_e = e1 - e0
                pi = np.zeros(M * 128, dtype=np.int64)
                pc = np.full(M * 128, -1.0, dtype=np.float32)
                pn = np.zeros(M * 128, dtype=np.float32)
                pi[:n_e] = rs[e0:e1] - (HALF if hf else 0)
                pc[:n_e] = (cs[e0:e1] - d0 - t * TW).astype(np.float32)
                pn[:n_e] = ns[e0:e1]
                ia[:, base * 8 : (base + M) * 8] = _wrap16(pi)
                # consumption-order columns: tile t chunk j -> ci_of_tile[t]+ (hf? Mlo[t]:0) + j
                cbase = int(ci_of_tile[t]) + (int(Mlo[t]) if hf else 0)
                colp_a[:, cbase : cbase + M] = pc.reshape(M, 128).T
                nrm_a[:, cbase : cbase + M] = pn.reshape(M, 128).T

        # add TW*batch-position offsets for the fused Idx comparison
        if USE_CUSTOM_DVE:
            # batches are per group, KB chunks each, offsets = TW * (pos in batch)
            for (g0, g1) in groups:
                ci0 = int(ci_of_tile[g0])
                ci1 = int(ci_of_tile[g1])
                n_cols = ci1 - ci0
                for b0 in range(0, n_cols, KB):
                    bk = min(KB, n_cols - b0)
                    offs = (np.arange(bk) * TW).astype(np.float32)
                    blk = colp_a[:, ci0 + b0 : ci0 + b0 + bk]
                    # padded (-1) stays out of range: -1 + TW*k never equals
                    # Idx in [TW*k, TW*(k+1)) except -1+TW*(k+1) = TW*k+TW-1-TW...
                    # careful: -1 + offs_next could alias; keep pads at -1e9
                    blk[blk < 0] = -1e9
                    blk += offs[None, :]
                    colp_a[:, ci0 + b0 : ci0 + b0 + bk] = blk

        in_maps.append(
            {
                "xlo": xlo,
                "xhi": xhi,
                "ilo": ilo_a,
                "ihi": ihi_a,
                "colp": colp_a,
                "nrm": nrm_a,
                "iot": iot,
                "x0t": np.ascontiguousarray(x0t).astype(bf),
                "wl": wl,
                "idm": idm,
                "yt": None,
            }
        )
        in_maps[-1].pop("yt")
    return schedule, in_maps


def kernel(x, x0, edge_index, norm, W):
    global LAST
    from concourse.bass_utils import run_bass_kernel_spmd

    schedule, in_maps = _preprocess(x, x0, edge_index, norm, W)
    key = (schedule["Mlo"], schedule["Mhi"], schedule["groups"])
    if key not in _prog_cache:
        _prog_cache[key] = _build_program(schedule)
    nc = _prog_cache[key]

    trace = os.environ.get("KERNEL_TRACE", "0") == "1"
    res = run_bass_kernel_spmd(
        nc,
        in_maps,
        core_ids=list(range(NCORES)),
        trace=trace,
    )
    LAST = res

    y = np.empty((N, D), dtype=np.float32)
    for c in range(NCORES):
        ytc = res.results[c]["yt"]
        d0 = c * NPC
        d1 = min(d0 + NPC, N)
        y[d0:d1] = ytc[:, : d1 - d0].T
    return y


# keep the reference-compatible signature name `TW` for kwargs call
def kernel_entry(**inputs):
    return kernel(
        inputs["x"], inputs["x0"], inputs["edge_index"], inputs["norm"], inputs["W"]
    )
